# revision 4
# baseline (speedup 1.0000x reference)
"""Trainium2 Bass kernel for the DTFA (dual-attention SE + threshold
decomposition) module — bf16 single-read version.

Math (per batch b):
  zt = SE(mean_T(x))            # [C, F]
  zf = SE(mean_F(x))            # [C, T]
  out1[t,f] = sum_c wf[c]*zf[c,t]*zt[c,f] + bf          (rank-C matmul)
  dcomp[k]  = where(out1 > thr_k, out1, 0), k=1..23
  out[c]    = (sum_k wf2[c,k]*dcomp[k] + bf2[c]) * x[c]

Sharding: pure data-parallel, 2 batches per core on 8 cores.

Strategy: the rel-err budget (2e-2) is spent on bf16 I/O. The host casts
the input to bf16; the kernel reads it ONCE into a resident SBUF cache
([128 (b,c) partitions, 65536 (t,f)] = 16 MB bf16), computes both SE
reductions from the cache (DVE innermost reduce for f-sums, bf16
accumulate+tree for t-sums), then pass 2 computes gates and multiplies
against the cached input, writing bf16 output that the host upcasts.
HBM traffic drops 100 MB -> 33.5 MB per core.
"""

import numpy as np

B, C, OC, T, F = 16, 64, 16, 256, 256
N_THR = 23
N_CORES = 8
BL = B // N_CORES          # local batches per core = 2
P = BL * C                 # 128 partitions = (b, c)
CH = 32                    # t-rows per pass-1 chunk
NCH = T // CH              # 8 chunks
WG = 16                    # t-rows per pass-2 output DMA
NWG = T // WG              # 16 groups

_CACHE = {}


def _host_constants(w1, b1, w2, b2, wf, bf, wf2, bf2):
    import ml_dtypes
    bfl = ml_dtypes.bfloat16
    f32 = np.float32
    w1 = np.asarray(w1, f32); b1 = np.asarray(b1, f32)
    w2 = np.asarray(w2, f32); b2 = np.asarray(b2, f32)
    wf = np.asarray(wf, f32); wf2 = np.asarray(wf2, f32)
    bf2 = np.asarray(bf2, f32)
    bf_s = float(np.asarray(bf).reshape(-1)[0])

    # SE weights, block-diagonal over the 2 local batches stacked on
    # partitions. 1/256 mean scale folded into w1.
    w1blk = np.zeros((P, 2 * OC), f32)
    w2blk = np.zeros((2 * OC, P), f32)
    for b in range(BL):
        w1blk[64 * b : 64 * b + 64, 16 * b : 16 * b + 16] = w1.T / 256.0
        w2blk[16 * b : 16 * b + 16, 64 * b : 64 * b + 64] = w2.T
    b1blk = np.tile(b1, BL).reshape(2 * OC, 1)
    b2blk = np.tile(b2, BL).reshape(P, 1)
    wfcol = np.tile(wf.reshape(-1), BL).reshape(P, 1)

    # Broadcast matmul weights: xB[m, n] = sum_k bcastW[k, m] * xflat[k, n]
    # xflat rows: 0 = A even-block pix, 1 = B even, 2 = A odd, 3 = B odd,
    # 4 = ones.  xB rows m: 0-47 even block (g=0), 64-111 odd (g=1);
    # within a 48-group: r = b*24 + k, k=0 the bias/ones row.
    bcastW = np.zeros((5, 112), f32)
    thrcol = np.zeros((112, 1), f32)
    for m2 in range(112):
        if 48 <= m2 < 64:
            thrcol[m2, 0] = 1e30
            continue
        g, r = (0, m2) if m2 < 48 else (1, m2 - 64)
        b_loc, k = divmod(r, 24)
        if k == 0:
            bcastW[4, m2] = 1.0
            thrcol[m2, 0] = -1e30
        else:
            bcastW[2 * g + b_loc, m2] = 1.0
            bcastW[4, m2] = bf_s
            thrcol[m2, 0] = k * (k + 1) / 600.0

    # Block-diagonal [bf2 | wf2] weights for the decomposition matmul.
    wbd = np.zeros((112, P), f32)
    for base in (0, 64):
        for b_loc in range(BL):
            for k in range(24):
                row = base + 24 * b_loc + k
                cols = slice(64 * b_loc, 64 * b_loc + 64)
                wbd[row, cols] = bf2 if k == 0 else wf2[:, k - 1]

    packF = np.zeros((128, 168), f32)
    packF[0:P, 0:32] = w1blk
    packF[0:32, 32:160] = w2blk
    packF[0:32, 160:161] = b1blk
    packF[0:P, 161:162] = b2blk
    packF[0:P, 162:163] = wfcol
    packF[0:112, 163:164] = thrcol
    packB = np.zeros((128, 248), bfl)
    packB[0:112, 0:128] = wbd.astype(bfl)
    packB[0:5, 128:240] = bcastW.astype(bfl)
    packB[0:112, 240:241] = thrcol.astype(bfl)
    return {
        "packF": packF,
        "packB": packB,
        "ones4k": np.ones((1, 4096), bfl),
    }


def _build_nc(reps=1, phase="all"):
    from contextlib import ExitStack, nullcontext

    import concourse.bass as bass
    import concourse.bacc as bacc
    import concourse.tile as tile
    from concourse import mybir

    f32 = mybir.dt.float32
    bf16 = mybir.dt.bfloat16
    Alu = mybir.AluOpType
    Act = mybir.ActivationFunctionType

    nc = bacc.Bacc("TRN2", target_bir_lowering=False, debug=False)
    feat = nc.dram_tensor("feat", [P, T, F], bf16, kind="ExternalInput")
    outp = nc.dram_tensor("outp", [P, T, F], bf16, kind="ExternalOutput")
    packFd = nc.dram_tensor("packF", [128, 168], f32, kind="ExternalInput")
    packBd = nc.dram_tensor("packB", [128, 248], bf16, kind="ExternalInput")
    ones4kd = nc.dram_tensor("ones4k", [1, 4096], bf16, kind="ExternalInput")

    with tile.TileContext(nc) as tc, ExitStack() as ctx:
        cpool = ctx.enter_context(tc.tile_pool(name="consts", bufs=1))
        cF = cpool.tile([128, 168], f32, tag="packF", name="c_packF")
        nc.gpsimd.dma_start(out=cF[:], in_=packFd[:])
        cB = cpool.tile([128, 248], bf16, tag="packB", name="c_packB")
        nc.gpsimd.dma_start(out=cB[:], in_=packBd[:])
        sb = {
            "w1blk": cF[0:P, 0:32], "w2blk": cF[0:32, 32:160],
            "b1blk": cF[0:32, 160:161], "b2blk": cF[0:P, 161:162],
            "wfcol": cF[0:P, 162:163], "thrF": cF[0:112, 163:164],
            "wbd": cB[0:112, 0:128], "bcastW": cB[0:5, 128:240],
            "thrB": cB[0:112, 240:241],
        }

        loop_cm = tc.For_i(0, reps, 1) if reps > 1 else nullcontext()
        ctx.enter_context(loop_cm)
        persist = ctx.enter_context(tc.tile_pool(name="persist", bufs=1))

        xc = persist.tile([P, T, F], bf16, tag="xc", name="xc")
        acc = persist.tile([P, CH, F], bf16, tag="acc", name="acc")
        zfs = persist.tile([P, T], f32, tag="zfs", name="zfs")
        zts = persist.tile([P, F], f32, tag="zts", name="zts")
        x_sb = persist.tile([128, 1024], bf16, tag="x_sb", name="x_sb")
        if phase == "p2":
            nc.gpsimd.memset(x_sb[:], 0.0)

        # ---------------- Pass 1: read-once + row/col sums ----------------
        for j in range(NCH if phase != "p2" else 0):
            dst = xc[:, CH * j : CH * j + CH, :]
            eng = nc.sync if j % 2 == 0 else nc.scalar
            eng.dma_start(out=dst, in_=feat[:, CH * j : CH * j + CH, :])
            nc.vector.tensor_reduce(
                out=zfs[:, CH * j : CH * j + CH], in_=dst,
                axis=mybir.AxisListType.X, op=Alu.add,
            )
            if j == 1:
                nc.vector.tensor_tensor(
                    out=acc[:], in0=xc[:, 0:CH, :], in1=dst, op=Alu.add)
            elif j > 1:
                nc.vector.tensor_tensor(
                    out=acc[:], in0=acc[:], in1=dst, op=Alu.add)
        if phase != "p2":
            # tree-fold acc [P, 32, F] -> zts [P, F] (f32 final)
            h = CH
            while h > 2:
                h //= 2
                nc.vector.tensor_tensor(
                    out=acc[:, 0:h, :], in0=acc[:, 0:h, :],
                    in1=acc[:, h : 2 * h, :], op=Alu.add)
            nc.vector.tensor_tensor(
                out=zts[:], in0=acc[:, 0, :], in1=acc[:, 1, :], op=Alu.add)

        # ---------------- SE branches + out1 ----------------
        sep = persist
        h1s = sep.tile([2 * OC, 256], f32, tag="h1s", name="h1s")
        ztg = sep.tile([P, 256], bf16, tag="ztg", name="ztg")
        zfg = sep.tile([P, 256], f32, tag="zfg", name="zfg")
        wfzf = sep.tile([P, 256], bf16, tag="wfzf", name="wfzf")
        with tc.tile_pool(name="ps_se", bufs=1, space="PSUM") as ppse:
            def se_h2(zin):
                h1 = ppse.tile([2 * OC, 256], f32, tag="h1")
                nc.tensor.matmul(h1[:], sb["w1blk"], zin)
                nc.scalar.activation(h1s[:], h1[:], Act.Relu,
                                     bias=sb["b1blk"], scale=1.0)
                h2 = ppse.tile([P, 256], f32, tag="h2")
                nc.tensor.matmul(h2[:], sb["w2blk"], h1s[:])
                return h2

            if phase != "p2":
                h2t = se_h2(zts[:])
                nc.scalar.activation(ztg[:], h2t[:], Act.Sigmoid,
                                     bias=sb["b2blk"], scale=1.0)
                h2f = se_h2(zfs[:])
                nc.scalar.activation(zfg[:], h2f[:], Act.Sigmoid,
                                     bias=sb["b2blk"], scale=1.0)
                nc.scalar.mul(wfzf[:], zfg[:], sb["wfcol"])
                for b in range(BL):
                    for m in range(2):
                        o1 = ppse.tile([128, 256], f32, tag="o1")
                        nc.tensor.matmul(
                            o1[:],
                            wfzf[64 * b : 64 * b + 64, 128 * m : 128 * m + 128],
                            ztg[64 * b : 64 * b + 64, :],
                        )
                        nc.scalar.copy(
                            x_sb[:, 256 * (2 * b + m) : 256 * (2 * b + m) + 256],
                            o1[:])

        # ---------------- x_flat gather: [5, 4096] per q ----------------
        # q covers pairs 8q..8q+7 (t-rows 32q..32q+31). Row layout:
        # 0 = A even blocks, 1 = B even, 2 = A odd, 3 = B odd, 4 = ones.
        if phase == "p1":
            xfpool = None
        else:
            xfpool = ctx.enter_context(tc.tile_pool(name="xflat", bufs=2))
        xflat = []
        for q in range(8 if phase != "p1" else 0):
            xf = xfpool.tile([5, 4096], bf16, tag="xf", name=f"xf{q}")
            m, tbase = divmod(q, 4)
            for par, (b_loc, off) in enumerate(
                [(0, 0), (1, 0), (0, 2), (1, 2)]
            ):
                srct = x_sb[:, 256 * (2 * b_loc + m) : 256 * (2 * b_loc + m) + 256]
                pitch = srct.ap[0][0]
                for sub in range(2):
                    row0 = 32 * tbase + off + sub
                    s0 = srct[row0 : row0 + 1, :]
                    src_ap = bass.AP(
                        tensor=s0.tensor, offset=s0.offset,
                        ap=[[4 * pitch, 8], [1, 256]],
                    )
                    d0 = xf[par : par + 1, :]
                    dst_ap = bass.AP(
                        tensor=d0.tensor, offset=d0.offset + 256 * sub,
                        ap=[[4096, 1], [512, 8], [1, 256]],
                    )
                    nc.gpsimd.dma_start(out=dst_ap, in_=src_ap)
            nc.gpsimd.dma_start(out=xf[4:5, :], in_=ones4kd[0:1, :])
            xflat.append(xf)

        # ---------------- Pass 2 ----------------
        opool = ctx.enter_context(tc.tile_pool(name="outs", bufs=2))
        xbspool = ctx.enter_context(tc.tile_pool(name="xbs", bufs=3))
        dcpool = ctx.enter_context(tc.tile_pool(name="dcomp", bufs=3))
        gspool = ctx.enter_context(tc.tile_pool(name="gs", bufs=3))
        ppxb = ctx.enter_context(tc.tile_pool(name="ps_xb", bufs=2, space="PSUM"))
        ppg = ctx.enter_context(tc.tile_pool(name="ps_g", bufs=2, space="PSUM"))

        for grp in range(NWG if phase == "all" else 0):
            ot = opool.tile([P, WG, F], bf16, tag="ot", name="ot")
            for ii in range(WG // 4):
                i = (WG // 4) * grp + ii
                q, r = divmod(i, 8)
                xB = ppxb.tile([112, 512], f32, tag="xB")
                nc.tensor.matmul(
                    xB[:], sb["bcastW"], xflat[q][:, 512 * r : 512 * r + 512]
                )
                xBs = xbspool.tile([112, 512], bf16, tag="xBs")
                nc.scalar.copy(xBs[:], xB[:])
                dc = dcpool.tile([112, 512], bf16, tag="dc")
                nc.vector.scalar_tensor_tensor(
                    out=dc[:], in0=xBs[:], scalar=sb["thrB"], in1=xBs[:],
                    op0=Alu.is_gt, op1=Alu.mult,
                )
                gp = ppg.tile([128, 1024], f32, tag="gp")
                for g in (0, 1):
                    nc.tensor.matmul(
                        gp[:, 512 * g : 512 * g + 512],
                        sb["wbd"][64 * g : 64 * g + 48, :],
                        dc[64 * g : 64 * g + 48, :],
                    )
                gs = gspool.tile([128, 1024], bf16, tag="gs")
                nc.scalar.copy(gs[:], gp[:])
                nc.vector.tensor_tensor(
                    out=ot[:, 4 * ii : 4 * ii + 4, :],
                    in0=gs[:].rearrange("p (a b) -> p a b", a=4),
                    in1=xc[:, 4 * i : 4 * i + 4, :], op=Alu.mult,
                )
            nc.scalar.dma_start(
                out=outp[:, WG * grp : WG * grp + WG, :], in_=ot[:]
            )

    nc.finalize()
    return nc


def _get_nc(reps=1, phase="all"):
    key = ("nc", reps, phase)
    if key not in _CACHE:
        _CACHE[key] = _build_nc(reps, phase)
    return _CACHE[key]


def _make_runner(nc, n_cores):
    """Cached jitted shard_map executor for `nc`."""
    import jax
    from jax.sharding import Mesh, PartitionSpec
    from jax.experimental.shard_map import shard_map
    from concourse import bass2jax, mybir

    bass2jax.install_neuronx_cc_hook()

    partition_name = (
        nc.partition_id_tensor.name if nc.partition_id_tensor else None
    )
    in_names, out_names, out_avals, zero_outs = [], [], [], []
    for alloc in nc.m.functions[0].allocations:
        if not isinstance(alloc, mybir.MemoryLocationSet):
            continue
        name = alloc.memorylocations[0].name
        if alloc.kind == "ExternalInput":
            if name != partition_name:
                in_names.append(name)
        elif alloc.kind == "ExternalOutput":
            out_names.append(name)
            shape = tuple(alloc.tensor_shape)
            dtype = mybir.dt.np(alloc.dtype)
            out_avals.append(jax.core.ShapedArray(shape, dtype))
            zero_outs.append(np.zeros(shape, dtype))
    n_params = len(in_names)
    all_in_names = in_names + out_names
    if partition_name is not None:
        all_in_names = all_in_names + [partition_name]
    donate = tuple(range(n_params, n_params + len(out_names)))

    def _body(*args):
        operands = list(args)
        if partition_name is not None:
            operands.append(bass2jax.partition_id_tensor())
        outs = bass2jax._bass_exec_p.bind(
            *operands,
            out_avals=tuple(out_avals),
            in_names=tuple(all_in_names),
            out_names=tuple(out_names),
            lowering_input_output_aliases=(),
            sim_require_finite=False,
            sim_require_nnan=False,
            nc=nc,
        )
        return tuple(outs)

    devices = jax.devices()[:n_cores]
    mesh = Mesh(np.asarray(devices), ("core",))
    specs = (PartitionSpec("core"),) * (n_params + len(out_names))
    sharded = jax.jit(
        shard_map(_body, mesh=mesh, in_specs=specs,
                  out_specs=(PartitionSpec("core"),) * len(out_names),
                  check_rep=False),
        donate_argnums=donate, keep_unused=True,
    )

    def run(in_maps):
        per_core = [[np.asarray(m[name]) for name in in_names] for m in in_maps]
        concat_in = [
            np.concatenate([per_core[c][i] for c in range(n_cores)], axis=0)
            for i in range(n_params)
        ]
        out_arrs = sharded(*concat_in, *[
            np.zeros((n_cores * z.shape[0], *z.shape[1:]), z.dtype)
            for z in zero_outs
        ])
        return [
            {
                name: np.asarray(out_arrs[i]).reshape(
                    n_cores, *out_avals[i].shape)[c]
                for i, name in enumerate(out_names)
            }
            for c in range(n_cores)
        ]

    run.sharded = sharded
    run.in_names = in_names
    run.out_names = out_names
    run.zero_outs = zero_outs
    run.n_params = n_params
    return run


def _get_runner(reps=1, phase="all"):
    key = ("runner", reps, phase)
    if key not in _CACHE:
        _CACHE[key] = _make_runner(_get_nc(reps, phase), N_CORES)
    return _CACHE[key]


def build_in_maps(inputs):
    """Per-core input dicts (bf16 feat + packed consts) from full inputs."""
    import ml_dtypes
    feature_in = np.asarray(inputs["feature_in"], np.float32)
    feat_bf = np.ascontiguousarray(
        feature_in.astype(ml_dtypes.bfloat16).reshape(B * C, T, F))
    consts = _host_constants(
        inputs["w1"], inputs["b1"], inputs["w2"], inputs["b2"],
        inputs["wf"], inputs["bf"], inputs["wf2"], inputs["bf2"],
    )
    in_maps = []
    for core in range(N_CORES):
        m = {"feat": feat_bf[P * core : P * core + P]}
        m.update(consts)
        in_maps.append(m)
    return in_maps


def kernel(**inputs):
    in_maps = build_in_maps(inputs)
    run = _get_runner()
    res = run(in_maps)
    out = np.concatenate(
        [np.asarray(res[c]["outp"], np.float32) for c in range(N_CORES)],
        axis=0)
    return out.reshape(B, C, T, F)
